# revision 6
# baseline (speedup 1.0000x reference)
"""ArcFace-style AngularPenaltySMLoss on 8 TRN2 NeuronCores.

Reference computation (N=4096, IN_F=512, OUT_F=20000, S=64, M=0.5):
    xn   = x / max(||x||_row, 1e-12)
    wf   = xn @ W.T                         # [N, OUT_F]
    diag = wf[i, labels[i]]
    num  = S*cos(acos(clip(diag)) + M)
    excl = sum_j exp(S*wf[i,j]) - exp(S*diag)
    L    = num - log(exp(num) + excl)
    out  = (-mean(L), wf)

Sharding: column-parallel over OUT_F (2500 cols/core, zero-padded to 2560).
Each core computes its [4096, 2500] slice of wf, the partial row sum-exp
over its columns, and the diag dot-products for its 512-row slice
(diag_i = xn_i . W[labels_i] is row-parallel).  Host does the O(N) scalar
epilogue and concatenates wf slices.

Device dataflow per core:
  - transposing DMA loads put the contraction dim (IN_F) on partitions:
      xT[k]  [128, 4096]  <- xn_bf16[:, 128k:128k+128].T
      wT[k]  [128, 2560]  <- w_bf16[:, 128k:128k+128].T
  - per (col-tile c, row-tile r): 4 accumulating bf16 matmuls -> PSUM f32
  - ScalarE: exp(S * psum) with fused accum_out -> partial row sum-exp
  - VectorE: copy psum -> SBUF f32, DMA out to wf
  - diag: tensor_tensor_reduce(xn_rows * wlab) row-sums
"""

import numpy as np
import ml_dtypes

from concourse import bass, bacc, tile, mybir
from concourse.bass_utils import run_bass_kernel_spmd

N, IN_F, OUT_F = 4096, 512, 20000
S, M, EPS = 64.0, 0.5, 1e-7
NCORES = 8
COLS = OUT_F // NCORES          # 2500 true columns per core
COLS_PAD = 2560                 # padded to 5 x 512 for uniform tiles
ROWS_PC = N // NCORES           # 512 diag rows per core
RT = N // 128                   # 32 row tiles
KT = IN_F // 128                # 4 contraction tiles
CT = COLS_PAD // 512            # 5 column tiles

_BF16 = mybir.dt.bfloat16
_F32 = mybir.dt.float32

_cached = {}


def _build_nc():
    nc = bacc.Bacc("TRN2", target_bir_lowering=False)

    xn_d = nc.declare_dram_parameter("xn", [N, IN_F], _BF16, isOutput=False)
    w_d = nc.declare_dram_parameter("w", [COLS_PAD, IN_F], _BF16, isOutput=False)
    xnr_d = nc.declare_dram_parameter("xnr", [ROWS_PC, IN_F], _F32, isOutput=False)
    wlab_d = nc.declare_dram_parameter("wlab", [ROWS_PC, IN_F], _F32, isOutput=False)
    wf_d = nc.declare_dram_parameter("wf", [N, COLS], _F32, isOutput=True)
    se_d = nc.declare_dram_parameter("se", [128, RT], _F32, isOutput=True)
    dg_d = nc.declare_dram_parameter("dg", [128, ROWS_PC // 128], _F32, isOutput=True)

    with tile.TileContext(nc) as tc:
        with (
            tc.tile_pool(name="big", bufs=1) as big,
            tc.tile_pool(name="work", bufs=4) as work,
            tc.tile_pool(name="scr", bufs=2) as scr,
            tc.tile_pool(name="acc", bufs=1) as accp,
            tc.tile_pool(name="psum", bufs=4, space="PSUM") as psum_pool,
        ):
            # --- transposing loads: put IN_F on partitions -----------------
            xT = [
                big.tile([128, N], _BF16, tag=f"xt{k}", name=f"xt{k}")
                for k in range(KT)
            ]
            wT = [
                big.tile([128, COLS_PAD], _BF16, tag=f"wt{k}", name=f"wt{k}")
                for k in range(KT)
            ]
            for k in range(KT):
                nc.sync.dma_start(
                    out=xT[k][:], in_=xn_d[:, k * 128:(k + 1) * 128], transpose=True
                )
                nc.sync.dma_start(
                    out=wT[k][:], in_=w_d[:, k * 128:(k + 1) * 128], transpose=True
                )

            # --- accumulators ---------------------------------------------
            se_acc = accp.tile([128, RT], _F32)
            nc.vector.memset(se_acc[:], 0.0)

            # --- main loop: wf tiles + exp row-sums ------------------------
            for c in range(CT):
                wcols = min(512, COLS - c * 512)  # 512,512,512,512,452
                for r in range(RT):
                    pt = psum_pool.tile([128, 512], _F32)
                    for k in range(KT):
                        nc.tensor.matmul(
                            pt[:],
                            xT[k][:, r * 128:(r + 1) * 128],
                            wT[k][:, c * 512:(c + 1) * 512],
                            start=(k == 0),
                            stop=(k == KT - 1),
                        )
                    # exp(S*wf) with fused row-sum into [128,1]
                    et = scr.tile([128, 512], _F32, tag="et")
                    st = scr.tile([128, 1], _F32, tag="st")
                    nc.scalar.activation(
                        et[:, :wcols],
                        pt[:, :wcols],
                        mybir.ActivationFunctionType.Exp,
                        scale=S,
                        accum_out=st[:],
                    )
                    nc.vector.tensor_add(
                        se_acc[:, r:r + 1], se_acc[:, r:r + 1], st[:]
                    )
                    # evacuate wf tile: PSUM -> SBUF -> DRAM
                    ot = work.tile([128, 512], _F32, tag="ot")
                    nc.vector.tensor_copy(ot[:, :wcols], pt[:, :wcols])
                    nc.sync.dma_start(
                        out=wf_d[r * 128:(r + 1) * 128, c * 512:c * 512 + wcols],
                        in_=ot[:, :wcols],
                    )

            nc.sync.dma_start(out=se_d[:], in_=se_acc[:])

            # --- diag: rowsum(xn_rows * wlab) ------------------------------
            dgt = accp.tile([128, ROWS_PC // 128], _F32)
            for t in range(ROWS_PC // 128):
                xnt = scr.tile([128, IN_F], _F32, tag="xnt")
                wlt = scr.tile([128, IN_F], _F32, tag="wlt")
                prod = scr.tile([128, IN_F], _F32, tag="prod")
                nc.sync.dma_start(out=xnt[:], in_=xnr_d[t * 128:(t + 1) * 128, :])
                nc.sync.dma_start(out=wlt[:], in_=wlab_d[t * 128:(t + 1) * 128, :])
                nc.vector.tensor_mul(prod[:], xnt[:], wlt[:])
                nc.vector.tensor_reduce(
                    dgt[:, t:t + 1],
                    prod[:],
                    axis=mybir.AxisListType.X,
                    op=mybir.AluOpType.add,
                )
            nc.sync.dma_start(out=dg_d[:], in_=dgt[:])

    nc.compile()
    return nc


def _make_in_maps(x, labels, W):
    """Host prologue: exact f32 normalization (matches reference), bf16
    casts for the matmul operands, W[labels] row gather, per-core shards."""
    x = np.asarray(x, dtype=np.float32)
    W = np.asarray(W, dtype=np.float32)
    labels = np.asarray(labels).astype(np.int64)

    norm = np.maximum(
        np.sqrt(np.einsum("ij,ij->i", x, x, dtype=np.float32)), np.float32(1e-12)
    )
    xn32 = x / norm[:, None].astype(np.float32)
    xn_bf = xn32.astype(ml_dtypes.bfloat16)
    W_bf = W.astype(ml_dtypes.bfloat16)
    Wlab = np.ascontiguousarray(W[labels])  # [N, IN_F] f32 row gather

    in_maps = []
    for c in range(NCORES):
        wsh = np.zeros((COLS_PAD, IN_F), dtype=ml_dtypes.bfloat16)
        wsh[:COLS] = W_bf[c * COLS:(c + 1) * COLS]
        in_maps.append(
            {
                "xn": xn_bf,
                "w": wsh,
                "xnr": np.ascontiguousarray(xn32[c * ROWS_PC:(c + 1) * ROWS_PC]),
                "wlab": np.ascontiguousarray(Wlab[c * ROWS_PC:(c + 1) * ROWS_PC]),
            }
        )
    return in_maps


def kernel(x, labels, W):
    if "nc" not in _cached:
        _cached["nc"] = _build_nc()
    nc = _cached["nc"]

    in_maps = _make_in_maps(x, labels, W)
    res = run_bass_kernel_spmd(nc, in_maps, core_ids=list(range(NCORES))).results

    # host epilogue: gather/unshard + O(N) scalar tail
    wf = np.empty((N, OUT_F), dtype=np.float32)
    se = np.zeros(N, dtype=np.float64)
    dg = np.empty(N, dtype=np.float32)
    for c in range(NCORES):
        wf[:, c * COLS:(c + 1) * COLS] = res[c]["wf"]
        se += res[c]["se"].T.reshape(-1).astype(np.float64)
        dg[c * ROWS_PC:(c + 1) * ROWS_PC] = res[c]["dg"].T.reshape(-1)

    d64 = dg.astype(np.float64)
    dc = np.clip(d64, -1.0 + EPS, 1.0 - EPS)
    numerator = S * (dc * np.cos(M) - np.sqrt(1.0 - dc * dc) * np.sin(M))
    excl = se - np.exp(S * d64)
    L = numerator - np.log(np.exp(numerator) + excl)
    loss = np.array(-np.mean(L), dtype=np.float32)
    return (loss, wf)


# revision 11
# speedup vs baseline: 1.2289x; 1.2289x over previous
"""ArcFace-style AngularPenaltySMLoss on 8 TRN2 NeuronCores.

Reference computation (N=4096, IN_F=512, OUT_F=20000, S=64, M=0.5):
    xn   = x / max(||x||_row, 1e-12)
    wf   = xn @ W.T                         # [N, OUT_F]
    diag = wf[i, labels[i]]
    num  = S*cos(acos(clip(diag)) + M)
    excl = sum_j exp(S*wf[i,j]) - exp(S*diag)
    L    = num - log(exp(num) + excl)
    out  = (-mean(L), wf)

Sharding: column-parallel over OUT_F (2500 cols/core, zero-padded to 2560).
Each core computes its [4096, 2500] slice of wf, the partial row sum-exp
over its columns, and the diag dot-products for its 512-row slice
(diag_i = xn_i . W[labels_i] is row-parallel).  Host does the O(N) scalar
epilogue and concatenates wf slices.

Device dataflow per core:
  - transposing DMA loads put the contraction dim (IN_F) on partitions:
      xT[k]  [128, 4096]  <- xn_bf16[:, 128k:128k+128].T
      wT[k]  [128, 2560]  <- w_bf16[:, 128k:128k+128].T
  - per (col-tile c, row-tile r): 4 accumulating bf16 matmuls -> PSUM f32
  - ScalarE: exp(S * psum) with fused accum_out -> partial row sum-exp
  - VectorE: copy psum -> SBUF f32, DMA out to wf
  - diag: tensor_tensor_reduce(xn_rows * wlab) row-sums
"""

import numpy as np
import ml_dtypes

from concourse import bass, bacc, tile, mybir
from concourse.bass_utils import run_bass_kernel_spmd

N, IN_F, OUT_F = 4096, 512, 20000
S, M, EPS = 64.0, 0.5, 1e-7
NCORES = 8
COLS = OUT_F // NCORES          # 2500 true columns per core
COLS_PAD = 2560                 # padded to 5 x 512 for uniform tiles
ROWS_PC = N // NCORES           # 512 diag rows per core
RT = N // 128                   # 32 row tiles
KT = IN_F // 128                # 4 contraction tiles
CT = COLS_PAD // 512            # 5 column tiles

_BF16 = mybir.dt.bfloat16
_F32 = mybir.dt.float32

_cached = {}


def _build_nc():
    nc = bacc.Bacc("TRN2", target_bir_lowering=False)

    xn_d = nc.declare_dram_parameter("xn", [N, IN_F], _BF16, isOutput=False)
    w_d = nc.declare_dram_parameter("w", [COLS_PAD, IN_F], _BF16, isOutput=False)
    xnr_d = nc.declare_dram_parameter("xnr", [ROWS_PC, IN_F], _F32, isOutput=False)
    wlab_d = nc.declare_dram_parameter("wlab", [ROWS_PC, IN_F], _F32, isOutput=False)
    wf_d = nc.declare_dram_parameter("wf", [N, COLS], _BF16, isOutput=True)
    # 3 exp-sum groups per row tile, summed on host
    se_d = nc.declare_dram_parameter("se", [128, RT * 3], _F32, isOutput=True)
    dg_d = nc.declare_dram_parameter("dg", [128, ROWS_PC // 128], _F32, isOutput=True)

    with tile.TileContext(nc) as tc:
        with (
            tc.tile_pool(name="big", bufs=1) as big,
            tc.tile_pool(name="work", bufs=4) as work,
            tc.tile_pool(name="scr", bufs=2) as scr,
            tc.tile_pool(name="acc", bufs=1) as accp,
            tc.tile_pool(name="psw", bufs=3, space="PSUM") as psw,
            tc.tile_pool(name="psn", bufs=2, space="PSUM") as psn,
        ):
            # --- transposing loads: put IN_F on partitions -----------------
            xT = [
                big.tile([128, N], _BF16, tag=f"xt{k}", name=f"xt{k}")
                for k in range(KT)
            ]
            wT = [
                big.tile([128, COLS_PAD], _BF16, tag=f"wt{k}", name=f"wt{k}")
                for k in range(KT)
            ]
            for k in range(KT):
                nc.sync.dma_start(
                    out=xT[k][:], in_=xn_d[:, k * 128:(k + 1) * 128], transpose=True
                )
                nc.sync.dma_start(
                    out=wT[k][:], in_=w_d[:, k * 128:(k + 1) * 128], transpose=True
                )

            # --- accumulator for per-group exp sums ------------------------
            se_acc = accp.tile([128, RT * 3], _F32)

            # --- main loop: one row tile at a time -------------------------
            # groups of PSUM banks: [2 banks, 2 banks, 1 bank] = 2560 cols
            groups = [(0, 1024, 1024), (1024, 1024, 1024), (2048, 512, 452)]
            for r in range(RT):
                ot = work.tile([128, COLS_PAD], _BF16, tag="ot")
                for gi, (goff, gw, gvalid) in enumerate(groups):
                    pool = psw if gw == 1024 else psn
                    pt = pool.tile(
                        [128, gw], _F32, tag=f"ptw" if gw == 1024 else "ptn",
                        name=f"pt{gi}_{r}",
                    )
                    for k in range(KT):
                        for ci in range(gw // 512):
                            nc.tensor.matmul(
                                pt[:, ci * 512:(ci + 1) * 512],
                                xT[k][:, r * 128:(r + 1) * 128],
                                wT[k][:, goff + ci * 512:goff + (ci + 1) * 512],
                                start=(k == 0),
                                stop=(k == KT - 1),
                            )
                    # exp(S*wf) with fused row-sum written straight to se_acc
                    et = scr.tile([128, 1024], _F32, tag="et", name=f"et{r}_{gi}")
                    nc.scalar.activation(
                        et[:, :gvalid],
                        pt[:, :gvalid],
                        mybir.ActivationFunctionType.Exp,
                        scale=S,
                        accum_out=se_acc[:, r * 3 + gi:r * 3 + gi + 1],
                    )
                    # evacuate wf group: PSUM f32 -> SBUF bf16
                    nc.vector.tensor_copy(
                        ot[:, goff:goff + gvalid], pt[:, :gvalid]
                    )
                nc.sync.dma_start(
                    out=wf_d[r * 128:(r + 1) * 128, :], in_=ot[:, :COLS]
                )

            nc.sync.dma_start(out=se_d[:], in_=se_acc[:])

            # --- diag: rowsum(xn_rows * wlab) ------------------------------
            dgt = accp.tile([128, ROWS_PC // 128], _F32)
            for t in range(ROWS_PC // 128):
                xnt = scr.tile([128, IN_F], _F32, tag="xnt")
                wlt = scr.tile([128, IN_F], _F32, tag="wlt")
                prod = scr.tile([128, IN_F], _F32, tag="prod")
                nc.sync.dma_start(out=xnt[:], in_=xnr_d[t * 128:(t + 1) * 128, :])
                nc.sync.dma_start(out=wlt[:], in_=wlab_d[t * 128:(t + 1) * 128, :])
                nc.vector.tensor_mul(prod[:], xnt[:], wlt[:])
                nc.vector.tensor_reduce(
                    dgt[:, t:t + 1],
                    prod[:],
                    axis=mybir.AxisListType.X,
                    op=mybir.AluOpType.add,
                )
            nc.sync.dma_start(out=dg_d[:], in_=dgt[:])

    nc.compile()
    return nc


def _make_in_maps(x, labels, W):
    """Host prologue: exact f32 normalization (matches reference), bf16
    casts for the matmul operands, W[labels] row gather, per-core shards."""
    x = np.asarray(x, dtype=np.float32)
    W = np.asarray(W, dtype=np.float32)
    labels = np.asarray(labels).astype(np.int64)

    norm = np.maximum(
        np.sqrt(np.einsum("ij,ij->i", x, x, dtype=np.float32)), np.float32(1e-12)
    )
    xn32 = x / norm[:, None].astype(np.float32)
    xn_bf = xn32.astype(ml_dtypes.bfloat16)
    W_bf = W.astype(ml_dtypes.bfloat16)
    Wlab = np.ascontiguousarray(W[labels])  # [N, IN_F] f32 row gather

    in_maps = []
    for c in range(NCORES):
        wsh = np.zeros((COLS_PAD, IN_F), dtype=ml_dtypes.bfloat16)
        wsh[:COLS] = W_bf[c * COLS:(c + 1) * COLS]
        in_maps.append(
            {
                "xn": xn_bf,
                "w": wsh,
                "xnr": np.ascontiguousarray(xn32[c * ROWS_PC:(c + 1) * ROWS_PC]),
                "wlab": np.ascontiguousarray(Wlab[c * ROWS_PC:(c + 1) * ROWS_PC]),
            }
        )
    return in_maps


def kernel(x, labels, W):
    if "nc" not in _cached:
        _cached["nc"] = _build_nc()
    nc = _cached["nc"]

    in_maps = _make_in_maps(x, labels, W)
    res = run_bass_kernel_spmd(nc, in_maps, core_ids=list(range(NCORES))).results

    # host epilogue: gather/unshard + O(N) scalar tail
    wf = np.empty((N, OUT_F), dtype=np.float32)
    se = np.zeros(N, dtype=np.float64)
    dg = np.empty(N, dtype=np.float32)
    for c in range(NCORES):
        wf[:, c * COLS:(c + 1) * COLS] = res[c]["wf"]  # bf16 -> f32 cast
        se_c = res[c]["se"].astype(np.float64).reshape(128, RT, 3).sum(-1)
        se += se_c.T.reshape(-1)
        dg[c * ROWS_PC:(c + 1) * ROWS_PC] = res[c]["dg"].T.reshape(-1)

    d64 = dg.astype(np.float64)
    dc = np.clip(d64, -1.0 + EPS, 1.0 - EPS)
    numerator = S * (dc * np.cos(M) - np.sqrt(1.0 - dc * dc) * np.sin(M))
    excl = se - np.exp(S * d64)
    L = numerator - np.log(np.exp(numerator) + excl)
    loss = np.array(-np.mean(L), dtype=np.float32)
    return (loss, wf)
